# revision 7
# baseline (speedup 1.0000x reference)
"""Bidirectional GRU + attention pooling + linear head on 8 Trainium2 NeuronCores.

Strategy (data-parallel over (seq, time) chunks, two SPMD launches):

Launch A (scan): the 16 independent GRU scans (8 seqs x fwd/bwd) are split
into time-chunks of length L with a warmup of W steps (the GRU state
contracts fast: warmup error ~1e-7 at W=64, measured).  Cores 0-3 run the
forward direction, cores 4-7 the backward direction on time-reversed input
(identical program, different data).  Each core batches 64 chunks (2 groups
of 32) through one weight stream per step; the input projection (W_ih @ x_t)
is fused into the same PSUM accumulation as the recurrent matmul.

Host: regroups the per-chunk hidden states into per-time-window `pred`
slabs ([hf; hb] per t).

Launch B (attention): each core handles one window of 512 timesteps for all
8 sequences: squish = tanh(W_att @ pred), scores = v . squish, local
softmax partials (m, sumexp, sum e^{s-m} * (w_lin @ pred_t)).

Host: combines the 8 windows' softmax partials exactly and applies the final
softmax.  All heavy compute runs on-device.
"""

import sys
import numpy as np

sys.path.insert(0, "/opt/trn_rl_repo")

import concourse.bacc as bacc  # noqa: E402
import concourse.tile as tile  # noqa: E402
from concourse import mybir  # noqa: E402
from concourse.bass_utils import run_bass_kernel_spmd  # noqa: E402

F32 = mybir.dt.float32
F16 = mybir.dt.float16
AF = mybir.ActivationFunctionType

B, T, I, H, O = 8, 4096, 128, 256, 64
NG, BC, W, L = 2, 32, 64, 128  # groups, chunks/group, warmup, chunk len
S = W + L  # steps per core
RBLK = 64  # state ring block (W == RBLK so warmup fills exactly one block)
SBLK = 64  # x staging block
NSB = S // SBLK
WIN = T // 8  # attention window per core

_cache = {}


def _build_scan():
    nc = bacc.Bacc("TRN2", target_bir_lowering=False, debug=False, num_devices=8)
    xt = nc.dram_tensor("xt", [NG, BC, 128, S], F32, kind="ExternalInput")
    wc = nc.dram_tensor("wc", [128, 3, 6, 128], F16, kind="ExternalInput")
    bi = nc.dram_tensor("bi", [128, 8, BC], F32, kind="ExternalInput")
    mk = nc.dram_tensor("mk", [128, NG, 2, BC], F16, kind="ExternalInput")
    st = nc.dram_tensor("st", [128, NG, 2, BC, L], F16, kind="ExternalOutput")

    # psum slot -> contributing contraction chunks (0,1 = h halves, 2 = x)
    KCS = [(0, 1, 2), (0, 1, 2), (0, 1, 2), (0, 1, 2), (0, 1), (0, 1), (2,), (2,)]
    # psum slot -> gate-row block of the weight tensor
    WMT = [0, 1, 2, 3, 4, 5, 4, 5]

    with tile.TileContext(nc) as tc:
        with (
            tc.tile_pool(name="const", bufs=1) as cpool,
            tc.tile_pool(name="xstage", bufs=2) as xsp,
            tc.tile_pool(name="xblk", bufs=1) as xbp,
            tc.tile_pool(name="ring", bufs=2) as ringp,
            tc.tile_pool(name="gates", bufs=3) as gp,
            tc.tile_pool(name="psum", bufs=2, space="PSUM") as pp,
        ):
            wsb = cpool.tile([128, 3, 6, 128], F16)
            nc.sync.dma_start(out=wsb, in_=wc.ap())
            bsb = cpool.tile([128, 8, BC], F32)
            nc.sync.dma_start(out=bsb, in_=bi.ap())
            msb = cpool.tile([128, NG, 2, BC], F16)
            nc.sync.dma_start(out=msb, in_=mk.ap())

            xblks = []
            for blk in range(NSB):
                xb = xbp.tile([128, NG, BC, SBLK], F16, tag=f"xb{blk}")
                for g in range(NG):
                    xs = xsp.tile([128, BC, SBLK], F32, tag="xs")
                    src = xt.ap()[g].rearrange("b p s -> p b s")[
                        :, :, blk * SBLK : (blk + 1) * SBLK
                    ]
                    nc.sync.dma_start(out=xs, in_=src)
                    nc.vector.tensor_copy(xb[:, g], xs)
                xblks.append(xb)

            hprev = []
            for g in range(NG):
                hz = gp.tile([128, 2, BC], F16, tag=f"h0g{g}")
                nc.vector.memset(hz, 0.0)
                hprev.append(hz)

            ring_cur = [None] * NG
            for s in range(S):
                blk_i = s // RBLK
                col = s % RBLK
                for g in range(NG):
                    if col == 0:
                        rtile = ringp.tile([128, 2, BC, RBLK], F16, tag=f"ring{g}")
                        ring_cur[g] = rtile
                    hp = hprev[g]
                    if s == W:
                        hm = gp.tile([128, 2, BC], F16, tag=f"hmask{g}")
                        nc.vector.tensor_mul(hm, hp, msb[:, g])
                        hp = hm
                    ps = pp.tile([128, 8, BC], F32, tag=f"ps{g}")
                    xcol = xblks[s // SBLK][:, g, :, s % SBLK]
                    rhs_by_kc = (hp[:, 0], hp[:, 1], xcol)
                    for mt in range(8):
                        kcs = KCS[mt]
                        for i, kc in enumerate(kcs):
                            nc.tensor.matmul(
                                ps[:, mt],
                                wsb[:, kc, WMT[mt]],
                                rhs_by_kc[kc],
                                start=(i == 0),
                                stop=(i == len(kcs) - 1),
                            )
                    gg = gp.tile([128, 8, BC], F32, tag=f"gg{g}")
                    nc.vector.tensor_add(gg, ps, bsb)
                    rz = gp.tile([128, 4, BC], F32, tag=f"rz{g}")
                    nc.scalar.activation(rz, gg[:, 0:4], AF.Sigmoid)
                    t1 = gp.tile([128, 2, BC], F32, tag=f"t1g{g}")
                    nc.vector.tensor_mul(t1, rz[:, 0:2], gg[:, 4:6])
                    t2 = gp.tile([128, 2, BC], F32, tag=f"t2g{g}")
                    nc.vector.tensor_add(t2, t1, gg[:, 6:8])
                    nt = gp.tile([128, 2, BC], F32, tag=f"ng{g}")
                    nc.scalar.activation(nt, t2, AF.Tanh)
                    d = gp.tile([128, 2, BC], F32, tag=f"dg{g}")
                    nc.vector.tensor_sub(d, hp, nt)
                    e = gp.tile([128, 2, BC], F32, tag=f"eg{g}")
                    nc.vector.tensor_mul(e, rz[:, 2:4], d)
                    hnew = ring_cur[g][:, :, :, col]
                    nc.vector.tensor_add(hnew, nt, e)
                    hprev[g] = hnew
                    if col == RBLK - 1 and s >= W:
                        nc.sync.dma_start(
                            out=st.ap()[
                                :, g, :, :, (blk_i - 1) * RBLK : blk_i * RBLK
                            ],
                            in_=ring_cur[g],
                        )
    nc.compile()
    return nc


def _build_attn():
    nc = bacc.Bacc("TRN2", target_bir_lowering=False, debug=False, num_devices=8)
    pred = nc.dram_tensor("pred", [B, 128, 4, WIN], F16, kind="ExternalInput")
    watt = nc.dram_tensor("watt", [128, 4, 4, 128], F16, kind="ExternalInput")
    vatt = nc.dram_tensor("vatt", [128, 4], F16, kind="ExternalInput")
    wlt = nc.dram_tensor("wlt", [128, 4, O], F16, kind="ExternalInput")
    idn = nc.dram_tensor("idn", [128, 128], F32, kind="ExternalInput")
    om = nc.dram_tensor("om", [B, 1], F32, kind="ExternalOutput")
    osm = nc.dram_tensor("osm", [B, 1], F32, kind="ExternalOutput")
    ou = nc.dram_tensor("ou", [O, B], F32, kind="ExternalOutput")

    NT = WIN // 128  # t-tiles per window

    with tile.TileContext(nc) as tc:
        with (
            tc.tile_pool(name="const", bufs=1) as cpool,
            tc.tile_pool(name="seq", bufs=2) as sqp,
            tc.tile_pool(name="acc", bufs=1) as acc,
            tc.tile_pool(name="ps_q", bufs=2, space="PSUM") as psq,
            tc.tile_pool(name="ps_s", bufs=1, space="PSUM") as pss,
            tc.tile_pool(name="ps_y", bufs=1, space="PSUM") as psy,
        ):
            wsb = cpool.tile([128, 4, 4, 128], F16)
            nc.sync.dma_start(out=wsb, in_=watt.ap())
            vsb = cpool.tile([128, 4], F16)
            nc.sync.dma_start(out=vsb, in_=vatt.ap())
            lsb = cpool.tile([128, 4, O], F16)
            nc.sync.dma_start(out=lsb, in_=wlt.ap())
            isb = cpool.tile([128, 128], F32)
            nc.sync.dma_start(out=isb, in_=idn.ap())

            scores = acc.tile([B, WIN], F32)
            ybig = acc.tile([128, B, NT, O], F16)

            for b in range(B):
                pb = sqp.tile([128, 4, WIN], F16, tag="pb")
                nc.sync.dma_start(out=pb, in_=pred.ap()[b])
                sq = sqp.tile([128, 4, WIN], F16, tag="sq")
                for kM in range(4):
                    qp = psq.tile([128, WIN], F32, tag="qp")
                    for hK in range(4):
                        nc.tensor.matmul(
                            qp,
                            wsb[:, hK, kM],
                            pb[:, hK],
                            start=(hK == 0),
                            stop=(hK == 3),
                        )
                    nc.scalar.activation(sq[:, kM], qp, AF.Tanh)
                sp = pss.tile([1, WIN], F32, tag="sp")
                for kM in range(4):
                    nc.tensor.matmul(
                        sp,
                        vsb[:, kM : kM + 1],
                        sq[:, kM],
                        start=(kM == 0),
                        stop=(kM == 3),
                    )
                srow = sqp.tile([1, WIN], F32, tag="srow")
                nc.vector.tensor_copy(srow, sp)
                nc.sync.dma_start(out=scores[b : b + 1], in_=srow)
                yp = psy.tile([128, NT, O], F32, tag="yp")
                for tt in range(NT):
                    for hK in range(4):
                        nc.tensor.matmul(
                            yp[:, tt],
                            pb[:, hK, tt * 128 : (tt + 1) * 128],
                            lsb[:, hK],
                            start=(hK == 0),
                            stop=(hK == 3),
                        )
                nc.vector.tensor_copy(ybig[:, b], yp)

            m = acc.tile([B, 1], F32)
            nc.vector.reduce_max(m, scores, axis=mybir.AxisListType.X)
            negm = acc.tile([B, 1], F32)
            nc.scalar.mul(negm, m, -1.0)
            ssum = acc.tile([B, 1], F32)
            ew = acc.tile([B, WIN], F32)
            nc.scalar.activation(ew, scores, AF.Exp, bias=negm, accum_out=ssum)
            ewt = acc.tile([128, NT, B], F16)
            for tt in range(NT):
                tp = pss.tile([128, B], F32, tag="tp")
                nc.tensor.transpose(tp, ew[:, tt * 128 : (tt + 1) * 128], isb[:B, :B])
                nc.vector.tensor_copy(ewt[:, tt], tp)
            usb = acc.tile([O, B], F32)
            for b in range(B):
                up = psy.tile([O, 1], F32, tag="up")
                for tt in range(NT):
                    nc.tensor.matmul(
                        up,
                        ybig[:, b, tt],
                        ewt[:, tt, b : b + 1],
                        start=(tt == 0),
                        stop=(tt == NT - 1),
                    )
                nc.vector.tensor_copy(usb[:, b : b + 1], up)
            nc.sync.dma_start(out=om.ap(), in_=m)
            nc.sync.dma_start(out=osm.ap(), in_=ssum)
            nc.sync.dma_start(out=ou.ap(), in_=usb)
    nc.compile()
    return nc


def _get(name, builder):
    if name not in _cache:
        _cache[name] = builder()
    return _cache[name]


def _mk_wc(w_ih, w_hh):
    wc = np.empty((128, 3, 6, 128), np.float16)
    whh = w_hh.reshape(6, 128, 2, 128)  # [mt, m, kc, p]
    wc[:, 0:2] = whh.transpose(3, 2, 0, 1)
    wc[:, 2] = w_ih.reshape(6, 128, 128).transpose(2, 0, 1)
    return wc


def _mk_bias(b_ih, b_hh):
    bia = np.empty((128, 8), np.float32)
    bia[:, 0:4] = (b_ih + b_hh)[:512].reshape(4, 128).T
    bia[:, 4:6] = b_hh[512:].reshape(2, 128).T
    bia[:, 6:8] = b_ih[512:].reshape(2, 128).T
    return np.repeat(bia[:, :, None], BC, axis=2).copy()



def run_scan_only(inputs):
    x = np.asarray(inputs["x"], np.float32)
    w_ih_f = np.asarray(inputs["w_ih_f"], np.float32)
    w_hh_f = np.asarray(inputs["w_hh_f"], np.float32)
    b_ih_f = np.asarray(inputs["b_ih_f"], np.float32)
    b_hh_f = np.asarray(inputs["b_hh_f"], np.float32)
    w_ih_b = np.asarray(inputs["w_ih_b"], np.float32)
    w_hh_b = np.asarray(inputs["w_hh_b"], np.float32)
    b_ih_b = np.asarray(inputs["b_ih_b"], np.float32)
    b_hh_b = np.asarray(inputs["b_hh_b"], np.float32)
    scan_nc = _get("scan", _build_scan)
    xT = np.ascontiguousarray(x.transpose(0, 2, 1))  # [B, I, T]
    xrevT = np.ascontiguousarray(x[:, ::-1, :].transpose(0, 2, 1))
    padf = np.zeros((B, I, W), np.float32)
    xpadT = np.concatenate([padf, xT], axis=2)
    xrevpadT = np.concatenate([padf, xrevT], axis=2)

    wcf, wcb = _mk_wc(w_ih_f, w_hh_f), _mk_wc(w_ih_b, w_hh_b)
    bif, bib = _mk_bias(b_ih_f, b_hh_f), _mk_bias(b_ih_b, b_hh_b)

    in_maps = []
    for r in range(8):
        rr = r % 4
        fwd = r < 4
        src = xpadT if fwd else xrevpadT
        xtc = np.empty((NG, BC, 128, S), np.float32)
        for g in range(NG):
            for c in range(BC):
                j = 8 * rr + g * 4 + c // 8
                b = c % 8
                xtc[g, c] = src[b, :, 128 * j : 128 * j + S]
        mask = np.ones((128, NG, 2, BC), np.float16)
        if rr == 0:
            mask[:, 0, :, 0:8] = 0.0
        in_maps.append(
            {
                "xt": xtc,
                "wc": wcf if fwd else wcb,
                "bi": bif if fwd else bib,
                "mk": mask,
            }
        )
    res_a = run_bass_kernel_spmd(scan_nc, in_maps, core_ids=list(range(8)))

    # ---- regroup states ----
    hf = np.empty((B, 2, 128, T), np.float16)
    hb_rev = np.empty((B, 2, 128, T), np.float16)
    for r in range(8):
        stt = res_a.results[r]["st"]  # [128, NG, 2, BC, L]
        rr = r % 4
        arr = stt.reshape(128, 2, 2, 4, 8, L)  # [p, g, ht, cj, b, l]
        arr = arr.transpose(4, 2, 0, 1, 3, 5).reshape(B, 2, 128, 8 * L)
        if r < 4:
            hf[:, :, :, 1024 * rr : 1024 * (rr + 1)] = arr
        else:
            hb_rev[:, :, :, 1024 * rr : 1024 * (rr + 1)] = arr
    hb = hb_rev[:, :, :, ::-1]
    return hf, hb


def kernel(**inputs):
    x = np.asarray(inputs["x"], np.float32)
    w_ih_f = np.asarray(inputs["w_ih_f"], np.float32)
    w_hh_f = np.asarray(inputs["w_hh_f"], np.float32)
    b_ih_f = np.asarray(inputs["b_ih_f"], np.float32)
    b_hh_f = np.asarray(inputs["b_hh_f"], np.float32)
    w_ih_b = np.asarray(inputs["w_ih_b"], np.float32)
    w_hh_b = np.asarray(inputs["w_hh_b"], np.float32)
    b_ih_b = np.asarray(inputs["b_ih_b"], np.float32)
    b_hh_b = np.asarray(inputs["b_hh_b"], np.float32)
    w_att = np.asarray(inputs["w_att"], np.float32)
    v_att = np.asarray(inputs["v_att"], np.float32)
    w_lin = np.asarray(inputs["w_lin"], np.float32)
    b_lin = np.asarray(inputs["b_lin"], np.float32)

    attn_nc = _get("attn", _build_attn)
    hf, hb = run_scan_only(inputs)


    # ---- launch B host prep ----
    wattp = np.ascontiguousarray(
        w_att.reshape(4, 128, 4, 128).transpose(1, 0, 2, 3)
    ).astype(np.float16)  # [p, hK, kM, m]
    vattp = np.ascontiguousarray(v_att[:, 0].reshape(4, 128).T).astype(np.float16)
    wltp = np.ascontiguousarray(
        w_lin.T.reshape(4, 128, O).transpose(1, 0, 2)
    ).astype(np.float16)  # [p, hK, o]
    eye = np.eye(128, dtype=np.float32)

    in_maps_b = []
    for c in range(8):
        tsl = slice(WIN * c, WIN * (c + 1))
        pc = np.empty((B, 128, 4, WIN), np.float16)
        pc[:, :, 0:2] = hf[:, :, :, tsl].transpose(0, 2, 1, 3)
        pc[:, :, 2:4] = hb[:, :, :, tsl].transpose(0, 2, 1, 3)
        in_maps_b.append(
            {"pred": pc, "watt": wattp, "vatt": vattp, "wlt": wltp, "idn": eye}
        )
    res_b = run_bass_kernel_spmd(attn_nc, in_maps_b, core_ids=list(range(8)))

    # ---- exact cross-window softmax combine on host ----
    ms = np.stack([res_b.results[c]["om"][:, 0] for c in range(8)])  # [8, B]
    ss = np.stack([res_b.results[c]["osm"][:, 0] for c in range(8)])  # [8, B]
    us = np.stack([res_b.results[c]["ou"] for c in range(8)])  # [8, O, B]
    mg = ms.max(0)  # [B]
    wgt = np.exp(ms - mg)  # [8, B]
    stot = (ss * wgt).sum(0)  # [B]
    uu = (us * wgt[:, None, :]).sum(0)  # [O, B]
    logits = (uu / stot).T + b_lin  # [B, O]
    z = logits - logits.max(1, keepdims=True)
    ez = np.exp(z)
    return (ez / ez.sum(1, keepdims=True)).astype(np.float32)
